# revision 14
# baseline (speedup 1.0000x reference)
"""Trainium2 Bass kernel for the CML1D problem — hybrid PE/vector version.

Math: 15 steps of  u' = bdm - (q0*s[i-1] + q1*s[i] + q2*s[i+1]),  s = u^2,
where u = g - 0.5 and bdm = beta*drive + 0.25*(q0+q1+q2) - 0.5.

Two independent layouts split the lattice so every engine stays busy:
  A-path (lat [0, LA)): lattice on partitions in shrink-windows of 128
    (halo 15 each side, stride 98 — no mid-iteration halo exchanges; the
    window's valid region shrinks by 1 row per step). Per step:
    psum = W^T s (PE, banded fp32), u = psum + bdm (GPSIMD stt),
    s' = u^2 (ACT Square).
  B-path (lat [LA, L)): batch on partitions (2 groups of 128 rows side by
    side on the free dim), lattice on the free dim in shrink-chunks
    (halo 15). The 3-tap conv is shifted-AP reads: 3 stt ops (DVE/GPSIMD)
    + ACT Square. No PE, no matmul.
PE (fp32 matmul, 4cyc/col) and DVE+GPSIMD+ACT are balanced by the LA/LB
ratio. Blocks/chunks are emitted in interleaved pairs so per-step serial
chains (mm->stt->sq) pipeline across the pair.
"""
import sys

sys.path.insert(0, "/opt/trn_rl_repo")
from contextlib import ExitStack

import numpy as np

import concourse.tile as tile
from concourse import bacc, mybir
from concourse.bass_utils import run_bass_kernel_spmd

F32 = mybir.dt.float32
BF16 = mybir.dt.bfloat16
AF = mybir.ActivationFunctionType
OP = mybir.AluOpType

R, EPS, BETA, STEPS = 3.9, 0.3, 0.15, 15
CLIP_LO, CLIP_HI = 0.0001, 1.0 - 0.0001

L = 16384
BATCH = 2048
N_CORES = 8
BPC = BATCH // N_CORES       # 256 rows per core

HALO = STEPS                 # 15: shrink-halo per side
WIN = 128
SA = WIN - 2 * HALO          # 98 valid lattice per A-window
NW = 64                      # windows -> LA = 6272
LA = NW * SA
LB = L - LA                  # 10112
NCH = 4                      # B chunks
WC = LB // NCH               # 1156 valid lattice per chunk
WCH = WC + 2 * HALO          # 1186 incl halo
NGRP = 2                     # batch row groups of 128 in B layout

BB = 32                      # batch rows per A block
NBLK = BPC // BB             # 8
CPB = NW * BB                # 3008 cols per A block
GROUP = 2048                 # psum drain group (4 banks)
MMN = 512                    # max fp32 moving free dim per matmul

assert NW * SA == LA and NCH * WC == LB and NBLK * BB == BPC


def build_nc():
    nc = bacc.Bacc("TRN2", target_bir_lowering=False, debug=False)
    drive_w = nc.dram_tensor("drive_w", [NBLK, WIN, CPB], F32, kind="ExternalInput")
    bdmh_w = nc.dram_tensor("bdmh_w", [NBLK, WIN, CPB], BF16, kind="ExternalInput")
    bdml_w = nc.dram_tensor("bdml_w", [NBLK, WIN, CPB], BF16, kind="ExternalInput")
    idmat = nc.dram_tensor("idmat", [WIN, WIN], BF16, kind="ExternalInput")
    drive_b = nc.dram_tensor("drive_b", [NCH, WIN, NGRP, WCH], F32, kind="ExternalInput")
    wmat = nc.dram_tensor("wmat", [WIN, WIN], F32, kind="ExternalInput")
    consts = nc.dram_tensor("consts", [WIN, 6], F32, kind="ExternalInput")
    out_a = nc.dram_tensor("out_a", [NBLK, SA, CPB], F32, kind="ExternalOutput")
    out_b = nc.dram_tensor("out_b", [NCH, WIN, NGRP, WC], F32, kind="ExternalOutput")

    # A psum groups
    a_groups = []
    g0 = 0
    while g0 < CPB:
        a_groups.append((g0, min(g0 + GROUP, CPB)))
        g0 = min(g0 + GROUP, CPB)

    # greedy D/G load balancer (ns per width unit: DVE 1/0.96, gpsimd 1/0.72)
    load = {"d": 0.0, "g": 0.0}

    def pick_dg(nc, width):
        if load["d"] / 0.96 + width / 0.96 <= load["g"] / 0.72 + width / 0.72:
            load["d"] += width
            return nc.vector
        load["g"] += width
        return nc.gpsimd

    with tile.TileContext(nc) as tc, ExitStack() as ctx:
        cp = ctx.enter_context(tc.tile_pool(name="constp", bufs=1))
        ap_s = ctx.enter_context(tc.tile_pool(name="a_state", bufs=2))
        ap_ld = ctx.enter_context(tc.tile_pool(name="a_ld", bufs=4))
        ap_bp = ctx.enter_context(tc.tile_pool(name="a_bdm", bufs=4))
        ap_u = ctx.enter_context(tc.tile_pool(name="a_u", bufs=2))
        bp_s = ctx.enter_context(tc.tile_pool(name="b_state", bufs=4))
        bp_ld = ctx.enter_context(tc.tile_pool(name="b_ld", bufs=5))
        bp_u = ctx.enter_context(tc.tile_pool(name="b_u", bufs=6))
        pp = ctx.enter_context(tc.tile_pool(name="ps", bufs=2, space="PSUM"))

        w_t = cp.tile([WIN, WIN], F32, tag="w")
        nc.sync.dma_start(w_t[:], wmat.ap())
        id_t = cp.tile([WIN, WIN], BF16, tag="idm")
        nc.sync.dma_start(id_t[:], idmat.ap())
        c_t = cp.tile([WIN, 6], F32, tag="c")
        nc.sync.dma_start(c_t[:], consts.ap())
        beta_ap = c_t[:, 0:1]
        csum_ap = c_t[:, 1:2]
        r0n_ap = c_t[:, 2:3]
        r2n_ap = c_t[:, 3:4]
        sq1_ap = c_t[:, 4:5]
        nhsq_ap = c_t[:, 5:6]
        neg_half = cp.tile([WIN, 1], F32, tag="nh")
        nc.vector.memset(neg_half[:], -0.5)

        # --- per-block / per-chunk state setup ---
        def a_load(blk):
            ld = ap_ld.tile([WIN, CPB], F32, tag="ld", name=f"a_ld{blk}")
            nc.sync.dma_start(ld[:], drive_w.ap()[blk])
            bh = ap_bp.tile([WIN, CPB], BF16, tag="bh", name=f"a_bh{blk}")
            bl = ap_bp.tile([WIN, CPB], BF16, tag="bl", name=f"a_bl{blk}")
            nc.sync.dma_start(bh[:], bdmh_w.ap()[blk])
            nc.sync.dma_start(bl[:], bdml_w.ap()[blk])
            return ld, bh, bl

        def a_derive(blk, pack):
            ld, bh, bl = pack
            s = ap_s.tile([WIN, CPB], F32, tag="s", name=f"a_s{blk}")
            nc.scalar.activation(s[:], ld[:], AF.Square, bias=neg_half[:], scale=1.0)
            return s, (bh, bl)

        def b_load(ch):
            ld = bp_ld.tile([WIN, NGRP, WCH], F32, tag="ld", name=f"b_ld{ch}")
            nc.sync.dma_start(ld[:], drive_b.ap()[ch])
            return ld

        def b_derive(ch, ld):
            s = bp_s.tile([WIN, NGRP, WCH], F32, tag="s", name=f"b_s{ch}")
            bdm = bp_s.tile([WIN, NGRP, WCH], F32, tag="bdm", name=f"b_bdm{ch}")
            # p = q1*s = Square(sqrt(q1)*d - 0.5*sqrt(q1))
            nc.scalar.activation(s[:], ld[:], AF.Square, bias=nhsq_ap, scale=sq1_ap)
            nc.gpsimd.tensor_scalar(bdm[:], ld[:], beta_ap, csum_ap, op0=OP.mult, op1=OP.add)
            return s, bdm

        # --- per-step bodies ---
        def a_step(blk, t, s, bdm):
            last = t == STEPS - 1
            bh, bl = bdm
            for gi, (g0, g1) in enumerate(a_groups):
                gw = g1 - g0
                pt = pp.tile([WIN, GROUP], F32, tag="ps", name=f"pt{t}_{gi}_b{blk}")
                m0 = 0
                while m0 < gw:
                    m1 = min(m0 + MMN, gw)
                    nc.tensor.matmul(
                        pt[:, m0:m1], w_t[:], s[:, g0 + m0 : g0 + m1],
                        start=True, stop=False,
                    )
                    m0 = m1
                # accumulate bdm = bh + bl into the same psum via bf16
                # identity matmuls (exact to ~2^-17; GPSIMD/DMA cannot reach
                # PSUM so this keeps the combine off the vector engines)
                m0 = 0
                while m0 < gw:
                    m1 = min(m0 + MMN, gw)
                    nc.tensor.matmul(
                        pt[:, m0:m1], id_t[:], bh[:, g0 + m0 : g0 + m1],
                        start=False, stop=False,
                    )
                    nc.tensor.matmul(
                        pt[:, m0:m1], id_t[:], bl[:, g0 + m0 : g0 + m1],
                        start=False, stop=(m1 == gw),
                    )
                    m0 = m1
                if not last:
                    nc.scalar.activation(
                        s[:, g0:g1], pt[:, :gw], AF.Square, bias=0.0, scale=1.0
                    )
                else:
                    o1 = ap_u.tile([WIN, GROUP], F32, tag="u", name=f"o1_{gi}_b{blk}")
                    nc.scalar.activation(
                        o1[:, :gw], pt[:, :gw], AF.Copy, bias=0.5, scale=1.0
                    )
                    o2 = ap_u.tile([WIN, GROUP], F32, tag="u", name=f"o2_{gi}_b{blk}")
                    nc.vector.tensor_scalar(
                        o2[:, :gw], o1[:, :gw], CLIP_LO, CLIP_HI, op0=OP.max, op1=OP.min
                    )
                    nc.sync.dma_start(
                        out_a.ap()[blk, :, g0:g1], o2[HALO : HALO + SA, :gw]
                    )

        def b_step(ch, t, s, bdm):
            last = t == STEPS - 1
            v0, v1 = t, WCH - t          # valid input cols
            w_out = v1 - v0 - 2
            a1 = bp_u.tile([WIN, NGRP, WCH], F32, tag="bu", name=f"a1_{t}_c{ch}")
            nc.vector.scalar_tensor_tensor(
                a1[:, :, : w_out], s[:, :, v0 : v1 - 2], r0n_ap,
                bdm[:, :, v0 + 1 : v1 - 1], op0=OP.mult, op1=OP.add,
            )
            a2 = bp_u.tile([WIN, NGRP, WCH], F32, tag="bu", name=f"a2_{t}_c{ch}")
            nc.vector.scalar_tensor_tensor(
                a2[:, :, : w_out], s[:, :, v0 + 2 : v1], r2n_ap,
                a1[:, :, : w_out], op0=OP.mult, op1=OP.add,
            )
            ut = bp_u.tile([WIN, NGRP, WCH], F32, tag="bu", name=f"bu_{t}_c{ch}")
            # u = a2 - p[0]  (center tap; p = q1*s so no scalar needed). Pool
            # runs TT at 0.42 efficiency, so a third of these go to DVE to
            # equalize the two engines.
            tt_eng = nc.vector if (ch + t) % 3 == 0 else nc.gpsimd
            tt_eng.tensor_tensor(
                ut[:, :, : w_out], a2[:, :, : w_out], s[:, :, v0 + 1 : v1 - 1],
                op=OP.subtract,
            )
            if not last:
                nc.scalar.activation(
                    s[:, :, v0 + 1 : v1 - 1], ut[:, :, : w_out],
                    AF.Square, bias=0.0, scale=sq1_ap,
                )
            else:
                o1 = bp_u.tile([WIN, NGRP, WCH], F32, tag="bu", name=f"bo1_c{ch}")
                nc.gpsimd.tensor_scalar(
                    o1[:, :, :WC], ut[:, :, : w_out], 0.5, CLIP_LO,
                    op0=OP.add, op1=OP.max,
                )
                o2 = bp_u.tile([WIN, NGRP, WCH], F32, tag="bu", name=f"bo2_c{ch}")
                nc.gpsimd.tensor_scalar(o2[:, :, :WC], o1[:, :, :WC], CLIP_HI, None, op0=OP.min)
                nc.sync.dma_start(out_b.ap()[ch], o2[:, :, :WC])

        # --- emission: paired blocks/chunks, step-interleaved ---
        n_rounds = NBLK // 2
        bpr = NCH // n_rounds       # B chunks per round (4)
        apack = {b: a_load(b) for b in (0, 1)}
        bpack = {c: b_load(c) for c in range(bpr)}
        for rnd in range(n_rounds):
            blocks = (2 * rnd, 2 * rnd + 1)
            chs = list(range(bpr * rnd, bpr * (rnd + 1)))
            astate = {b: a_derive(b, apack.pop(b)) for b in blocks}
            bstate = {c: b_derive(c, bpack.pop(c)) for c in chs}
            if rnd + 1 < n_rounds:
                # prefetch next round's inputs while this round computes
                for b in (2 * rnd + 2, 2 * rnd + 3):
                    apack[b] = a_load(b)
                for c in range(bpr * (rnd + 1), bpr * (rnd + 2)):
                    bpack[c] = b_load(c)
            for t in range(STEPS):
                for b in blocks:
                    a_step(b, t, *astate[b])
                for c in chs:
                    b_step(c, t, *bstate[c])

    nc.compile()
    return nc


def _host_constants(K):
    K = np.asarray(K, dtype=np.float64)
    q0 = (1.0 - BETA) * EPS * K[0] * R
    q1 = (1.0 - BETA) * (1.0 - EPS + EPS * K[1]) * R
    q2 = (1.0 - BETA) * EPS * K[2] * R
    W = np.zeros((WIN, WIN), np.float32)
    for p in range(WIN):
        if p >= 1:
            W[p - 1, p] = -q0
        W[p, p] = -q1
        if p + 1 < WIN:
            W[p + 1, p] = -q2
    csum = 0.25 * (q0 + q1 + q2) - 0.5
    sq1 = np.sqrt(q1)
    consts = np.empty((WIN, 6), np.float32)
    consts[:, 0] = BETA
    consts[:, 1] = csum
    consts[:, 2] = -q0 / q1
    consts[:, 3] = -q2 / q1
    consts[:, 4] = sq1
    consts[:, 5] = -0.5 * sq1
    return W, consts


def _window_a(d):
    """[BPC, L] -> [NBLK, WIN, CPB]: shrink-windows stride SA, halo HALO."""
    c_idx = np.arange(NW) * SA
    p_idx = np.arange(WIN)
    idx = (c_idx[:, None] + p_idx[None, :] - HALO) % L  # [NW, WIN]
    win = d[:, idx]  # [BPC, NW, WIN]
    win = win.reshape(NBLK, BB, NW, WIN).transpose(0, 3, 2, 1)
    return np.ascontiguousarray(win).reshape(NBLK, WIN, CPB)


def _unwindow_a(o):
    """[NBLK, SA, CPB] -> [BPC, LA]."""
    o = o.reshape(NBLK, SA, NW, BB).transpose(0, 3, 2, 1)  # [NBLK, BB, NW, SA]
    return o.reshape(BPC, NW * SA)


def _window_b(d):
    """[BPC, L] -> [NCH, WIN, NGRP, WCH]."""
    out = np.empty((NCH, WIN, NGRP, WCH), np.float32)
    for c in range(NCH):
        lat = (LA + c * WC - HALO + np.arange(WCH)) % L
        seg = d[:, lat]  # [BPC, WCH]
        out[c] = seg.reshape(NGRP, WIN, WCH).transpose(1, 0, 2)
    return np.ascontiguousarray(out)


def _unwindow_b(o):
    """[NCH, WIN, NGRP, WC] -> [BPC, LB] (lat LA..L)."""
    return o.transpose(2, 1, 0, 3).reshape(BPC, NCH * WC)


_NC_CACHE = {}
TRACE = False
LAST_RESULT = None


def _get_nc(*_a):
    if "nc" not in _NC_CACHE:
        _NC_CACHE["nc"] = build_nc()
    return _NC_CACHE["nc"]


def kernel(drive, K):
    drive = np.asarray(drive, dtype=np.float32)
    K = np.asarray(K, dtype=np.float32)
    b, mid, lat = drive.shape
    d2 = drive.reshape(b, lat)
    W, consts = _host_constants(K)
    nc = _get_nc()

    import ml_dtypes

    idm = np.eye(WIN, dtype=ml_dtypes.bfloat16)
    in_maps = []
    for c in range(N_CORES):
        dcore = d2[c * BPC : (c + 1) * BPC]
        dw = _window_a(dcore)
        bdm = (np.float32(consts[0, 0]) * dw + np.float32(consts[0, 1])).astype(
            np.float32
        )
        bh = bdm.astype(ml_dtypes.bfloat16)
        bl = (bdm - bh.astype(np.float32)).astype(ml_dtypes.bfloat16)
        in_maps.append(
            {
                "drive_w": dw,
                "bdmh_w": bh,
                "bdml_w": bl,
                "idmat": idm,
                "drive_b": _window_b(dcore),
                "wmat": W,
                "consts": consts,
            }
        )
    global LAST_RESULT
    res = None
    for attempt in range(3):
        try:
            res = run_bass_kernel_spmd(
                nc, in_maps, core_ids=list(range(N_CORES)), trace=TRACE
            )
            break
        except Exception:
            if attempt == 2:
                raise
            import time

            try:
                import jax

                jax.clear_caches()
                from jax._src import xla_bridge

                xla_bridge._clear_backends()
            except Exception:
                pass
            time.sleep(5.0)
    LAST_RESULT = res
    outs = []
    for c in range(N_CORES):
        oa = _unwindow_a(res.results[c]["out_a"])  # [BPC, LA]
        ob = _unwindow_b(res.results[c]["out_b"])  # [BPC, LB]
        outs.append(np.concatenate([oa, ob], axis=1))
    out = np.concatenate(outs, axis=0).reshape(b, mid, lat).astype(np.float32)
    return out


# revision 18
# speedup vs baseline: 1.1270x; 1.1270x over previous
"""Trainium2 Bass kernel for the CML1D problem — hybrid PE/vector version.

Math: 15 steps of  u' = bdm - (q0*s[i-1] + q1*s[i] + q2*s[i+1]),  s = u^2,
where u = g - 0.5 and bdm = beta*drive + 0.25*(q0+q1+q2) - 0.5.

Two independent layouts split the lattice so every engine stays busy:
  A-path (lat [0, LA)): lattice on partitions in shrink-windows of 128
    (halo 15 each side, stride 98 — no mid-iteration halo exchanges; the
    window's valid region shrinks by 1 row per step). Per step:
    psum = W^T s (PE, banded fp32), u = psum + bdm (GPSIMD stt),
    s' = u^2 (ACT Square).
  B-path (lat [LA, L)): batch on partitions (2 groups of 128 rows side by
    side on the free dim), lattice on the free dim in shrink-chunks
    (halo 15). The 3-tap conv is shifted-AP reads: 3 stt ops (DVE/GPSIMD)
    + ACT Square. No PE, no matmul.
PE (fp32 matmul, 4cyc/col) and DVE+GPSIMD+ACT are balanced by the LA/LB
ratio. Blocks/chunks are emitted in interleaved pairs so per-step serial
chains (mm->stt->sq) pipeline across the pair.
"""
import sys

sys.path.insert(0, "/opt/trn_rl_repo")
from contextlib import ExitStack

import numpy as np

import concourse.tile as tile
from concourse import bacc, mybir
from concourse.bass_utils import run_bass_kernel_spmd

F32 = mybir.dt.float32
BF16 = mybir.dt.bfloat16
AF = mybir.ActivationFunctionType
OP = mybir.AluOpType

R, EPS, BETA, STEPS = 3.9, 0.3, 0.15, 15
CLIP_LO, CLIP_HI = 0.0001, 1.0 - 0.0001

L = 16384
BATCH = 2048
N_CORES = 8
BPC = BATCH // N_CORES       # 256 rows per core

HALO = STEPS                 # 15: shrink-halo per side
WIN = 128
SA = WIN - 2 * HALO          # 98 valid lattice per A-window
NW = 64                      # windows -> LA = 6272
LA = NW * SA
LB = L - LA                  # 10112
NCH = 4                      # B chunks
WC = LB // NCH               # 1156 valid lattice per chunk
WCH = WC + 2 * HALO          # 1186 incl halo
NGRP = 2                     # batch row groups of 128 in B layout

BB = 32                      # batch rows per A block
NBLK = BPC // BB             # 8
CPB = NW * BB                # 3008 cols per A block
GROUP = 2048                 # psum drain group (4 banks)
MMN = 512                    # max fp32 moving free dim per matmul

assert NW * SA == LA and NCH * WC == LB and NBLK * BB == BPC


def build_nc():
    nc = bacc.Bacc("TRN2", target_bir_lowering=False, debug=False)
    drive_w = nc.dram_tensor("drive_w", [NBLK, WIN, CPB], F32, kind="ExternalInput")
    bdmh_w = nc.dram_tensor("bdmh_w", [NBLK, WIN, CPB], BF16, kind="ExternalInput")
    bdml_w = nc.dram_tensor("bdml_w", [NBLK, WIN, CPB], BF16, kind="ExternalInput")
    idmat = nc.dram_tensor("idmat", [WIN, WIN], BF16, kind="ExternalInput")
    drive_b = nc.dram_tensor("drive_b", [NCH, WIN, NGRP, WCH], F32, kind="ExternalInput")
    wmat = nc.dram_tensor("wmat", [WIN, WIN], F32, kind="ExternalInput")
    consts = nc.dram_tensor("consts", [WIN, 6], F32, kind="ExternalInput")
    out_a = nc.dram_tensor("out_a", [NBLK, SA, CPB], F32, kind="ExternalOutput")
    out_b = nc.dram_tensor("out_b", [NCH, WIN, NGRP, WC], F32, kind="ExternalOutput")

    # A psum groups
    a_groups = []
    g0 = 0
    while g0 < CPB:
        a_groups.append((g0, min(g0 + GROUP, CPB)))
        g0 = min(g0 + GROUP, CPB)

    # greedy D/G load balancer (ns per width unit: DVE 1/0.96, gpsimd 1/0.72)
    load = {"d": 0.0, "g": 0.0}

    def pick_dg(nc, width):
        if load["d"] / 0.96 + width / 0.96 <= load["g"] / 0.72 + width / 0.72:
            load["d"] += width
            return nc.vector
        load["g"] += width
        return nc.gpsimd

    with tile.TileContext(nc) as tc, ExitStack() as ctx:
        cp = ctx.enter_context(tc.tile_pool(name="constp", bufs=1))
        ap_s = ctx.enter_context(tc.tile_pool(name="a_state", bufs=2))
        ap_ld = ctx.enter_context(tc.tile_pool(name="a_ld", bufs=4))
        ap_bp = ctx.enter_context(tc.tile_pool(name="a_bdm", bufs=4))
        ap_u = ctx.enter_context(tc.tile_pool(name="a_u", bufs=2))
        bp_s = ctx.enter_context(tc.tile_pool(name="b_state", bufs=4))
        bp_ld = ctx.enter_context(tc.tile_pool(name="b_ld", bufs=5))
        bp_u = ctx.enter_context(tc.tile_pool(name="b_u", bufs=6))
        pp = ctx.enter_context(tc.tile_pool(name="ps", bufs=2, space="PSUM"))

        w_t = cp.tile([WIN, WIN], F32, tag="w")
        nc.sync.dma_start(w_t[:], wmat.ap())
        id_t = cp.tile([WIN, WIN], BF16, tag="idm")
        nc.sync.dma_start(id_t[:], idmat.ap())
        c_t = cp.tile([WIN, 6], F32, tag="c")
        nc.sync.dma_start(c_t[:], consts.ap())
        beta_ap = c_t[:, 0:1]
        csum_ap = c_t[:, 1:2]
        r0n_ap = c_t[:, 2:3]
        r2n_ap = c_t[:, 3:4]
        sq1_ap = c_t[:, 4:5]
        nhsq_ap = c_t[:, 5:6]
        neg_half = cp.tile([WIN, 1], F32, tag="nh")
        nc.vector.memset(neg_half[:], -0.5)

        # --- per-block / per-chunk state setup ---
        def a_load(blk):
            ld = ap_ld.tile([WIN, CPB], F32, tag="ld", name=f"a_ld{blk}")
            nc.sync.dma_start(ld[:], drive_w.ap()[blk])
            bh = ap_bp.tile([WIN, CPB], BF16, tag="bh", name=f"a_bh{blk}")
            bl = ap_bp.tile([WIN, CPB], BF16, tag="bl", name=f"a_bl{blk}")
            nc.sync.dma_start(bh[:], bdmh_w.ap()[blk])
            nc.sync.dma_start(bl[:], bdml_w.ap()[blk])
            return ld, bh, bl

        def a_derive(blk, pack):
            ld, bh, bl = pack
            s = ap_s.tile([WIN, CPB], F32, tag="s", name=f"a_s{blk}")
            nc.scalar.activation(s[:], ld[:], AF.Square, bias=neg_half[:], scale=1.0)
            return s, (bh, bl)

        def b_load(ch):
            ld = bp_ld.tile([WIN, NGRP, WCH], F32, tag="ld", name=f"b_ld{ch}")
            nc.sync.dma_start(ld[:], drive_b.ap()[ch])
            return ld

        def b_derive(ch, ld):
            s = bp_s.tile([WIN, NGRP, WCH], F32, tag="s", name=f"b_s{ch}")
            bdm = bp_s.tile([WIN, NGRP, WCH], F32, tag="bdm", name=f"b_bdm{ch}")
            # p = q1*s = Square(sqrt(q1)*d - 0.5*sqrt(q1))
            nc.scalar.activation(s[:], ld[:], AF.Square, bias=nhsq_ap, scale=sq1_ap)
            nc.gpsimd.tensor_scalar(bdm[:], ld[:], beta_ap, csum_ap, op0=OP.mult, op1=OP.add)
            return s, bdm

        # --- per-step bodies ---
        def a_step(blk, t, s, bdm):
            last = t == STEPS - 1
            bh, bl = bdm
            for gi, (g0, g1) in enumerate(a_groups):
                gw = g1 - g0
                pt = pp.tile([WIN, GROUP], F32, tag="ps", name=f"pt{t}_{gi}_b{blk}")
                # ~15% of block-steps use a DVE combine instead of the
                # identity matmuls, trading spare DVE time for PE time
                dve_combine = (not last) and ((blk * STEPS + t) % 10 == 0)
                m0 = 0
                while m0 < gw:
                    m1 = min(m0 + MMN, gw)
                    nc.tensor.matmul(
                        pt[:, m0:m1], w_t[:], s[:, g0 + m0 : g0 + m1],
                        start=True, stop=(dve_combine and m1 == gw),
                    )
                    m0 = m1
                if not dve_combine:
                    # accumulate bdm = bh + bl into the same psum via bf16
                    # identity matmuls (exact to ~2^-17; GPSIMD/DMA cannot
                    # reach PSUM so this keeps the combine off the vector
                    # engines)
                    m0 = 0
                    while m0 < gw:
                        m1 = min(m0 + MMN, gw)
                        nc.tensor.matmul(
                            pt[:, m0:m1], id_t[:], bh[:, g0 + m0 : g0 + m1],
                            start=False, stop=False,
                        )
                        nc.tensor.matmul(
                            pt[:, m0:m1], id_t[:], bl[:, g0 + m0 : g0 + m1],
                            start=False, stop=(m1 == gw),
                        )
                        m0 = m1
                if dve_combine:
                    u1 = ap_u.tile([WIN, GROUP], F32, tag="u", name=f"u1_{t}_b{blk}")
                    nc.vector.scalar_tensor_tensor(
                        u1[:, :gw], pt[:, :gw], 1.0, bh[:, g0:g1],
                        op0=OP.mult, op1=OP.add,
                    )
                    u2 = ap_u.tile([WIN, GROUP], F32, tag="u", name=f"u2_{t}_b{blk}")
                    nc.vector.scalar_tensor_tensor(
                        u2[:, :gw], u1[:, :gw], 1.0, bl[:, g0:g1],
                        op0=OP.mult, op1=OP.add,
                    )
                    nc.scalar.activation(
                        s[:, g0:g1], u2[:, :gw], AF.Square, bias=0.0, scale=1.0
                    )
                elif not last:
                    nc.scalar.activation(
                        s[:, g0:g1], pt[:, :gw], AF.Square, bias=0.0, scale=1.0
                    )
                else:
                    o1 = ap_u.tile([WIN, GROUP], F32, tag="u", name=f"o1_{gi}_b{blk}")
                    nc.scalar.activation(
                        o1[:, :gw], pt[:, :gw], AF.Copy, bias=0.5, scale=1.0
                    )
                    o2 = ap_u.tile([WIN, GROUP], F32, tag="u", name=f"o2_{gi}_b{blk}")
                    nc.vector.tensor_scalar(
                        o2[:, :gw], o1[:, :gw], CLIP_LO, CLIP_HI, op0=OP.max, op1=OP.min
                    )
                    nc.sync.dma_start(
                        out_a.ap()[blk, :, g0:g1], o2[HALO : HALO + SA, :gw]
                    )

        def b_step(ch, t, s, bdm):
            last = t == STEPS - 1
            v0, v1 = t, WCH - t          # valid input cols
            w_out = v1 - v0 - 2
            a1 = bp_u.tile([WIN, NGRP, WCH], F32, tag="bu", name=f"a1_{t}_c{ch}")
            nc.vector.scalar_tensor_tensor(
                a1[:, :, : w_out], s[:, :, v0 : v1 - 2], r0n_ap,
                bdm[:, :, v0 + 1 : v1 - 1], op0=OP.mult, op1=OP.add,
            )
            a2 = bp_u.tile([WIN, NGRP, WCH], F32, tag="bu", name=f"a2_{t}_c{ch}")
            nc.vector.scalar_tensor_tensor(
                a2[:, :, : w_out], s[:, :, v0 + 2 : v1], r2n_ap,
                a1[:, :, : w_out], op0=OP.mult, op1=OP.add,
            )
            ut = bp_u.tile([WIN, NGRP, WCH], F32, tag="bu", name=f"bu_{t}_c{ch}")
            # u = a2 - p[0]  (center tap; p = q1*s so no scalar needed -> Pool TT)
            nc.gpsimd.tensor_tensor(
                ut[:, :, : w_out], a2[:, :, : w_out], s[:, :, v0 + 1 : v1 - 1],
                op=OP.subtract,
            )
            if not last:
                nc.scalar.activation(
                    s[:, :, v0 + 1 : v1 - 1], ut[:, :, : w_out],
                    AF.Square, bias=0.0, scale=sq1_ap,
                )
            else:
                o1 = bp_u.tile([WIN, NGRP, WCH], F32, tag="bu", name=f"bo1_c{ch}")
                nc.gpsimd.tensor_scalar(
                    o1[:, :, :WC], ut[:, :, : w_out], 0.5, CLIP_LO,
                    op0=OP.add, op1=OP.max,
                )
                o2 = bp_u.tile([WIN, NGRP, WCH], F32, tag="bu", name=f"bo2_c{ch}")
                nc.gpsimd.tensor_scalar(o2[:, :, :WC], o1[:, :, :WC], CLIP_HI, None, op0=OP.min)
                nc.sync.dma_start(out_b.ap()[ch], o2[:, :, :WC])

        # --- emission: paired blocks/chunks, step-interleaved ---
        n_rounds = NBLK // 2
        bpr = NCH // n_rounds       # B chunks per round (4)
        apack = {b: a_load(b) for b in (0, 1)}
        bpack = {c: b_load(c) for c in range(bpr)}
        for rnd in range(n_rounds):
            blocks = (2 * rnd, 2 * rnd + 1)
            chs = list(range(bpr * rnd, bpr * (rnd + 1)))
            astate = {b: a_derive(b, apack.pop(b)) for b in blocks}
            bstate = {c: b_derive(c, bpack.pop(c)) for c in chs}
            if rnd + 1 < n_rounds:
                # prefetch next round's inputs while this round computes
                for b in (2 * rnd + 2, 2 * rnd + 3):
                    apack[b] = a_load(b)
                for c in range(bpr * (rnd + 1), bpr * (rnd + 2)):
                    bpack[c] = b_load(c)
            for t in range(STEPS):
                for b in blocks:
                    a_step(b, t, *astate[b])
                for c in chs:
                    b_step(c, t, *bstate[c])

    nc.compile()
    return nc


def _host_constants(K):
    K = np.asarray(K, dtype=np.float64)
    q0 = (1.0 - BETA) * EPS * K[0] * R
    q1 = (1.0 - BETA) * (1.0 - EPS + EPS * K[1]) * R
    q2 = (1.0 - BETA) * EPS * K[2] * R
    W = np.zeros((WIN, WIN), np.float32)
    for p in range(WIN):
        if p >= 1:
            W[p - 1, p] = -q0
        W[p, p] = -q1
        if p + 1 < WIN:
            W[p + 1, p] = -q2
    csum = 0.25 * (q0 + q1 + q2) - 0.5
    sq1 = np.sqrt(q1)
    consts = np.empty((WIN, 6), np.float32)
    consts[:, 0] = BETA
    consts[:, 1] = csum
    consts[:, 2] = -q0 / q1
    consts[:, 3] = -q2 / q1
    consts[:, 4] = sq1
    consts[:, 5] = -0.5 * sq1
    return W, consts


def _window_a(d):
    """[BPC, L] -> [NBLK, WIN, CPB]: shrink-windows stride SA, halo HALO."""
    c_idx = np.arange(NW) * SA
    p_idx = np.arange(WIN)
    idx = (c_idx[:, None] + p_idx[None, :] - HALO) % L  # [NW, WIN]
    win = d[:, idx]  # [BPC, NW, WIN]
    win = win.reshape(NBLK, BB, NW, WIN).transpose(0, 3, 2, 1)
    return np.ascontiguousarray(win).reshape(NBLK, WIN, CPB)


def _unwindow_a(o):
    """[NBLK, SA, CPB] -> [BPC, LA]."""
    o = o.reshape(NBLK, SA, NW, BB).transpose(0, 3, 2, 1)  # [NBLK, BB, NW, SA]
    return o.reshape(BPC, NW * SA)


def _window_b(d):
    """[BPC, L] -> [NCH, WIN, NGRP, WCH]."""
    out = np.empty((NCH, WIN, NGRP, WCH), np.float32)
    for c in range(NCH):
        lat = (LA + c * WC - HALO + np.arange(WCH)) % L
        seg = d[:, lat]  # [BPC, WCH]
        out[c] = seg.reshape(NGRP, WIN, WCH).transpose(1, 0, 2)
    return np.ascontiguousarray(out)


def _unwindow_b(o):
    """[NCH, WIN, NGRP, WC] -> [BPC, LB] (lat LA..L)."""
    return o.transpose(2, 1, 0, 3).reshape(BPC, NCH * WC)


_NC_CACHE = {}
TRACE = False
LAST_RESULT = None


def _get_nc(*_a):
    if "nc" not in _NC_CACHE:
        _NC_CACHE["nc"] = build_nc()
    return _NC_CACHE["nc"]


def kernel(drive, K):
    drive = np.asarray(drive, dtype=np.float32)
    K = np.asarray(K, dtype=np.float32)
    b, mid, lat = drive.shape
    d2 = drive.reshape(b, lat)
    W, consts = _host_constants(K)
    nc = _get_nc()

    import ml_dtypes

    idm = np.eye(WIN, dtype=ml_dtypes.bfloat16)
    in_maps = []
    for c in range(N_CORES):
        dcore = d2[c * BPC : (c + 1) * BPC]
        dw = _window_a(dcore)
        bdm = (np.float32(consts[0, 0]) * dw + np.float32(consts[0, 1])).astype(
            np.float32
        )
        bh = bdm.astype(ml_dtypes.bfloat16)
        bl = (bdm - bh.astype(np.float32)).astype(ml_dtypes.bfloat16)
        in_maps.append(
            {
                "drive_w": dw,
                "bdmh_w": bh,
                "bdml_w": bl,
                "idmat": idm,
                "drive_b": _window_b(dcore),
                "wmat": W,
                "consts": consts,
            }
        )
    global LAST_RESULT
    res = None
    for attempt in range(3):
        try:
            res = run_bass_kernel_spmd(
                nc, in_maps, core_ids=list(range(N_CORES)), trace=TRACE
            )
            break
        except Exception:
            if attempt == 2:
                raise
            import time

            try:
                import jax

                jax.clear_caches()
                from jax._src import xla_bridge

                xla_bridge._clear_backends()
            except Exception:
                pass
            time.sleep(5.0)
    LAST_RESULT = res
    outs = []
    for c in range(N_CORES):
        oa = _unwindow_a(res.results[c]["out_a"])  # [BPC, LA]
        ob = _unwindow_b(res.results[c]["out_b"])  # [BPC, LB]
        outs.append(np.concatenate([oa, ob], axis=1))
    out = np.concatenate(outs, axis=0).reshape(b, mid, lat).astype(np.float32)
    return out
